# revision 10
# baseline (speedup 1.0000x reference)
"""Trainium2 Bass kernel for nn_Attention_5643587027315 (8 NeuronCores).

Sharding: tensor-parallel over the 16 heads (2 heads per core).
Per core: qkv^T projection (d-major) -> RoPE -> flash-style attention with
kerple bias applied as a multiplicative table (1+a*d)^(-p) -> AllToAll of the
context (token resharding) -> token-sharded output projection.
"""
import numpy as np
import ml_dtypes

B, S, DM, H, D = 2, 2048, 1024, 16, 64
N = B * S            # 4096 flattened tokens
SCALE = 1.0 / float(np.sqrt(D))
bf16 = ml_dtypes.bfloat16

_GRAPH_CACHE = {}


def _build_graph():
    import concourse.bass as bass
    import concourse.mybir as mybir
    import concourse.tile as tile
    from concourse import bacc
    from concourse.masks import make_identity
    from contextlib import ExitStack

    F32 = mybir.dt.float32
    BF16 = mybir.dt.bfloat16

    nc = bacc.Bacc("TRN2", target_bir_lowering=False, num_devices=8)

    x_p = nc.declare_dram_parameter("x_bf", [N, DM], BF16, isOutput=False)
    wT_p = nc.declare_dram_parameter("wT", [DM, 384], BF16, isOutput=False)
    ow_p = nc.declare_dram_parameter("out_wT", [DM, DM], BF16, isOutput=False)
    cc_p = nc.declare_dram_parameter("cc", [128, N], BF16, isOutput=False)
    ss_p = nc.declare_dram_parameter("ssg", [128, N], BF16, isOutput=False)
    kt_p = nc.declare_dram_parameter("ktab", [2, 4100], BF16, isOutput=False)
    out_p = nc.declare_dram_parameter("out", [512, DM], F32, isOutput=True)

    Exp = mybir.ActivationFunctionType.Exp
    Recip = mybir.ActivationFunctionType.Reciprocal

    with tile.TileContext(nc) as tc, ExitStack() as top:
        const = top.enter_context(tc.tile_pool(name="const", bufs=1))
        qkv_sb = top.enter_context(tc.tile_pool(name="qkv_sb", bufs=1))
        ctxp = top.enter_context(tc.tile_pool(name="ctxp", bufs=1))
        dram = top.enter_context(tc.tile_pool(name="dram", bufs=1, space="DRAM"))

        w_sb = const.tile([128, 8, 384], BF16)
        cc_sb = const.tile([128, N], BF16)
        ss_sb = const.tile([128, N], BF16)
        ident = const.tile([128, 128], BF16)
        vt_sb = const.tile([128, 32, 130], BF16)

        make_identity(nc, ident)
        nc.sync.dma_start(out=w_sb, in_=wT_p.rearrange("(a p) r -> p a r", p=128))
        nc.sync.dma_start(out=cc_sb, in_=cc_p[:])
        nc.sync.dma_start(out=ss_sb, in_=ss_p[:])

        q_sb = qkv_sb.tile([128, N], BF16)
        k_sb = qkv_sb.tile([128, N], BF16)
        v_sb = qkv_sb.tile([128, N], BF16)
        ctx_h = [ctxp.tile([64, N], BF16, name=f"ctxu{i}") for i in range(2)]
        den_sb = [ctxp.tile([1, N], F32, name=f"den{i}") for i in range(2)]

        with ExitStack() as ph_a:
            xt_pool = ph_a.enter_context(tc.tile_pool(name="xt", bufs=8))
            qk_psum = ph_a.enter_context(
                tc.tile_pool(name="qk_psum", bufs=2, space="PSUM"))
            tp_psum = ph_a.enter_context(
                tc.tile_pool(name="tp_psum", bufs=2, space="PSUM"))
            rope_pool = ph_a.enter_context(tc.tile_pool(name="rope", bufs=1))

            xt = []
            for dk in range(8):
                t = xt_pool.tile([128, N], BF16, name=f"xt{dk}", tag="xt")
                nc.sync.dma_start(
                    out=t, in_=x_p[:, dk * 128:(dk + 1) * 128], transpose=True)
                xt.append(t)

            # qkv^T = wT.T @ xT   -> [384, 4096] (rows: q_h0,q_h1,k_h0,k_h1,v_h0,v_h1)
            dsts = (q_sb, k_sb, v_sb)
            for r3 in range(3):
                for tcx in range(8):
                    ps = qk_psum.tile([128, 512], F32, name="qkps")
                    for dk in range(8):
                        nc.tensor.matmul(
                            ps,
                            w_sb[:, dk, r3 * 128:(r3 + 1) * 128],
                            xt[dk][:, tcx * 512:(tcx + 1) * 512],
                            start=(dk == 0), stop=(dk == 7))
                    nc.any.tensor_copy(
                        dsts[r3][:, tcx * 512:(tcx + 1) * 512], ps)

            # RoPE on q and k (swap halves via sbuf->sbuf DMA, then 3 DVE ops)
            for X in (q_sb, k_sb):
                xsw = rope_pool.tile([128, N], BF16, name="xsw", tag="xsw")
                for (d0, s0) in ((0, 32), (32, 0), (64, 96), (96, 64)):
                    nc.sync.dma_start(
                        out=xsw[d0:d0 + 32, :], in_=X[s0:s0 + 32, :])
                a_t = rope_pool.tile([128, N], BF16, name="ropea", tag="ropea")
                nc.vector.tensor_mul(a_t, X, cc_sb)
                nc.vector.tensor_mul(xsw, xsw, ss_sb)
                nc.vector.tensor_add(X, a_t, xsw)

            # v token-major tiles with ones column (for softmax denominator)
            for ti in range(32):
                pst = tp_psum.tile([128, 128], BF16, name="tps")
                nc.tensor.transpose(
                    pst, v_sb[:, ti * 128:(ti + 1) * 128], ident)
                nc.any.tensor_copy(vt_sb[:, ti, 0:64], pst[:, 0:64])
                nc.any.tensor_copy(vt_sb[:, ti, 65:129], pst[:, 64:128])
            nc.vector.memset(vt_sb[:, :, 64], 1.0)
            nc.vector.memset(vt_sb[:, :, 129], 1.0)

        ow_pool = top.enter_context(tc.tile_pool(name="ow", bufs=1))
        ow_sb = ow_pool.tile([128, 8, DM], BF16)
        nc.sync.dma_start(out=ow_sb, in_=ow_p.rearrange("(a p) r -> p a r", p=128))

        # ---- attention ----
        with ExitStack() as ph_b:
            sc_psum = ph_b.enter_context(
                tc.tile_pool(name="sc_psum", bufs=2, space="PSUM"))
            ctx_psum = ph_b.enter_context(
                tc.tile_pool(name="ctx_psum", bufs=2, space="PSUM"))
            e_pool = ph_b.enter_context(tc.tile_pool(name="e_pool", bufs=3))
            w2_pool = ph_b.enter_context(tc.tile_pool(name="w2_pool", bufs=3))

            for qc2 in range(2):
                for hl in range(2):
                    h64 = hl * 64
                    acc = [ctx_psum.tile([65, 1024], F32, name=f"acc{b}",
                                         tag="acc") for b in range(B)]
                    for kt_i in range(16):
                        k0 = kt_i * 128
                        w2 = w2_pool.tile([128, 1032], BF16, name="w2", tag="w2")
                        off = hl * 4100 + k0 - 1024 * qc2 + 1024
                        src = bass.AP(tensor=kt_p, offset=off,
                                      ap=[[1, 128], [1, 1025]])
                        nc.sync.dma_start(out=w2[:, 0:1025], in_=src)
                        kview = w2[:, 1024:0:-1]
                        for b in range(B):
                            cb = b * S
                            ps = sc_psum.tile([128, 1024], F32, name="scps",
                                              tag="scps")
                            for half in range(2):
                                nc.tensor.matmul(
                                    ps[:, half * 512:(half + 1) * 512],
                                    k_sb[h64:h64 + 64, cb + k0: cb + k0 + 128],
                                    q_sb[h64:h64 + 64,
                                         cb + qc2 * 1024 + half * 512:
                                         cb + qc2 * 1024 + (half + 1) * 512],
                                    start=True, stop=True)
                            e = e_pool.tile([128, 1024], BF16, name="e", tag="e")
                            nc.scalar.activation(e, ps, Exp, scale=SCALE)
                            nc.vector.tensor_mul(e, e, kview)
                            gti = b * 16 + kt_i
                            for half in range(2):
                                nc.tensor.matmul(
                                    acc[b][:, half * 512:(half + 1) * 512],
                                    vt_sb[:, gti, hl * 65:hl * 65 + 65],
                                    e[:, half * 512:(half + 1) * 512],
                                    start=(kt_i == 0), stop=(kt_i == 15))
                    for b in range(B):
                        sl = slice(b * S + qc2 * 1024, b * S + qc2 * 1024 + 1024)
                        nc.any.tensor_copy(ctx_h[hl][:, sl], acc[b][0:64, :])
                        nc.any.tensor_copy(den_sb[hl][:, sl], acc[b][64:65, :])

        # ---- normalize (single table-switch to Reciprocal) ----
        with ExitStack() as ph_n:
            bc_psum = ph_n.enter_context(
                tc.tile_pool(name="bc_psum", bufs=2, space="PSUM"))
            div_pool = ph_n.enter_context(tc.tile_pool(name="div", bufs=2))
            ones64 = const.tile([1, 64], F32)
            nc.vector.memset(ones64, 1.0)
            ctxn = [ctxp.tile([64, N], BF16, name=f"ctxn{i}") for i in range(2)]
            for hl in range(2):
                rcp = div_pool.tile([1, N], F32, name="rcp", tag="rcp")
                nc.vector.reciprocal_approx_fast(rcp, den_sb[hl][:])
                rb_sb = div_pool.tile([64, N], F32, name="rbsb", tag="rbsb")
                for half in range(8):
                    rb_ps = bc_psum.tile([64, 512], F32, name="rbps", tag="rbps")
                    nc.tensor.matmul(
                        rb_ps, ones64, rcp[:, half * 512:(half + 1) * 512],
                        start=True, stop=True)
                    nc.any.tensor_copy(
                        rb_sb[:, half * 512:(half + 1) * 512], rb_ps)
                nc.vector.tensor_mul(ctxn[hl], ctx_h[hl][:], rb_sb)

        # ---- AllToAll: reshard from head-split to token-split ----
        a2a_in = dram.tile([8, 128, 512], BF16)
        a2a_out = dram.tile([8, 128, 512], BF16)
        for hl in range(2):
            nc.sync.dma_start(
                out=a2a_in[:, hl * 64:hl * 64 + 64, :].transpose([1, 0, 2]),
                in_=ctxn[hl].rearrange("p (j q) -> p j q", j=8))
        nc.gpsimd.collective_compute(
            "AllToAll", mybir.AluOpType.bypass,
            replica_groups=[list(range(8))],
            ins=[a2a_in.opt()], outs=[a2a_out.opt()])

        # ---- output projection for this core's 512-token slice ----
        with ExitStack() as ph_y:
            y_psum = ph_y.enter_context(
                tc.tile_pool(name="y_psum", bufs=2, space="PSUM"))
            y_lhs = ph_y.enter_context(tc.tile_pool(name="y_lhs", bufs=3))
            y_out = ph_y.enter_context(tc.tile_pool(name="y_out", bufs=2))
            for tt in range(4):
                ps_y = y_psum.tile([128, 1024], F32, name="psy", tag="psy")
                for j in range(8):
                    lh = y_lhs.tile([128, 128], BF16, name="ylh", tag="ylh")
                    nc.sync.dma_start(
                        out=lh, in_=a2a_out[j, :, tt * 128:(tt + 1) * 128])
                    for dmc in range(2):
                        nc.tensor.matmul(
                            ps_y[:, dmc * 512:(dmc + 1) * 512],
                            lh, ow_sb[:, j, dmc * 512:(dmc + 1) * 512],
                            start=(j == 0), stop=(j == 7))
                y_sb = y_out.tile([128, 1024], F32, name="ysb", tag="ysb")
                nc.any.tensor_copy(y_sb, ps_y)
                nc.sync.dma_start(
                    out=out_p[tt * 128:(tt + 1) * 128, :], in_=y_sb)

    nc.compile()
    return nc


def _host_prep(x, qkv_w, out_w, bias_p, bias_a, rope_freqs, c):
    x = np.asarray(x, np.float32).reshape(N, DM)
    qkv_w = np.asarray(qkv_w, np.float32)
    out_w = np.asarray(out_w, np.float32)
    bias_p = np.asarray(bias_p, np.float32).reshape(H)
    bias_a = np.asarray(bias_a, np.float32).reshape(H)
    freqs = np.asarray(rope_freqs, np.float32)

    h0, h1 = 2 * c, 2 * c + 1
    rows = []
    for blk in range(3):
        for h in (h0, h1):
            rows.append(qkv_w[blk * 1024 + h * 64: blk * 1024 + h * 64 + 64])
    wT = np.ascontiguousarray(np.concatenate(rows, 0).T)

    pos = np.arange(S, dtype=np.float32)
    ang = pos[:, None] * freqs[None, :]
    cosT = np.cos(ang).T
    sinT = np.sin(ang).T
    cc64 = np.concatenate([cosT, cosT], 0)
    ss64 = np.concatenate([-sinT, sinT], 0)
    CC = np.tile(np.concatenate([cc64, cc64], 0), (1, B))
    SSg = np.tile(np.concatenate([ss64, ss64], 0), (1, B))

    p = np.maximum(bias_p, 0.01)
    a = np.maximum(bias_a, 0.01)
    idx = np.abs(np.arange(4100, dtype=np.float32) - 2048.0)
    ktab = np.stack([(1.0 + a[h] * idx) ** (-p[h]) for h in (h0, h1)], 0)

    return {
        "x_bf": x.astype(bf16),
        "wT": wT.astype(bf16),
        "out_wT": np.ascontiguousarray(out_w.T).astype(bf16),
        "cc": np.ascontiguousarray(CC).astype(bf16),
        "ssg": np.ascontiguousarray(SSg).astype(bf16),
        "ktab": np.ascontiguousarray(ktab).astype(bf16),
    }


def kernel(x, qkv_w, out_w, bias_p, bias_a, rope_freqs, _trace=False):
    from concourse.bass_utils import run_bass_kernel_spmd

    if "nc" not in _GRAPH_CACHE:
        _GRAPH_CACHE["nc"] = _build_graph()
    nc = _GRAPH_CACHE["nc"]

    in_maps = [
        _host_prep(x, qkv_w, out_w, bias_p, bias_a, rope_freqs, c)
        for c in range(8)
    ]
    res = run_bass_kernel_spmd(nc, in_maps, core_ids=list(range(8)),
                               trace=_trace)
    _GRAPH_CACHE["last_result"] = res
    y = np.concatenate([np.asarray(res.results[c]["out"]) for c in range(8)], 0)
    return np.ascontiguousarray(y.reshape(B, S, DM)).astype(np.float32)


# revision 39
# speedup vs baseline: 74.8603x; 74.8603x over previous
"""Trainium2 Bass kernel for nn_Attention_5643587027315 (8 NeuronCores).

Sharding: tensor-parallel over the 16 heads (2 heads per core).
Per core: qkv^T projection (d-major) -> RoPE -> flash-style attention with
kerple bias applied as a multiplicative table (1+a*d)^(-p) -> AllToAll of the
context (token resharding) -> token-sharded output projection.
"""
import numpy as np
import ml_dtypes

B, S, DM, H, D = 2, 2048, 1024, 16, 64
N = B * S            # 4096 flattened tokens
SCALE = 1.0 / float(np.sqrt(D))
bf16 = ml_dtypes.bfloat16

_GRAPH_CACHE = {}


def _build_graph():
    import concourse.bass as bass
    import concourse.mybir as mybir
    import concourse.tile as tile
    from concourse import bacc
    from concourse.masks import make_identity
    from contextlib import ExitStack

    F32 = mybir.dt.float32
    BF16 = mybir.dt.bfloat16

    nc = bacc.Bacc("TRN2", target_bir_lowering=False, num_devices=8)

    x_p = nc.declare_dram_parameter("x_bf", [N, DM], BF16, isOutput=False)
    wT_p = nc.declare_dram_parameter("wT", [DM, 384], BF16, isOutput=False)
    ow_p = nc.declare_dram_parameter("out_wT", [DM, DM], BF16, isOutput=False)
    cc_p = nc.declare_dram_parameter("cc", [128, N], BF16, isOutput=False)
    ss_p = nc.declare_dram_parameter("ssg", [128, N], BF16, isOutput=False)
    kt_p = nc.declare_dram_parameter("ktab", [2, 4100], BF16, isOutput=False)
    out_p = nc.declare_dram_parameter("out", [512, DM], F32, isOutput=True)

    Exp = mybir.ActivationFunctionType.Exp
    Recip = mybir.ActivationFunctionType.Reciprocal

    with tile.TileContext(nc) as tc, ExitStack() as top:
        const = top.enter_context(tc.tile_pool(name="const", bufs=1))
        qkv_sb = top.enter_context(tc.tile_pool(name="qkv_sb", bufs=1))
        ctxp = top.enter_context(tc.tile_pool(name="ctxp", bufs=1))
        dram = top.enter_context(tc.tile_pool(name="dram", bufs=1, space="DRAM"))

        w_sb = const.tile([128, 8, 384], BF16)
        cc_sb = const.tile([128, N], BF16)
        ss_sb = const.tile([128, N], BF16)
        ident = const.tile([128, 128], BF16)
        vt_c = [const.tile([128, 8, 130], BF16, name=f"vt{i}")
                for i in range(4)]

        make_identity(nc, ident)
        nc.sync.dma_start(out=w_sb, in_=wT_p.rearrange("(a p) r -> p a r", p=128))

        q_c = [qkv_sb.tile([128, 1024], BF16, name=f"qc{i}") for i in range(4)]
        k_c = [qkv_sb.tile([128, 1024], BF16, name=f"kc{i}") for i in range(4)]
        v_c = [qkv_sb.tile([128, 1024], BF16, name=f"vc{i}") for i in range(4)]

        with ExitStack() as ph_a:
            xt_pool = ph_a.enter_context(tc.tile_pool(name="xt", bufs=8))
            qk_psum = ph_a.enter_context(
                tc.tile_pool(name="qk_psum", bufs=6, space="PSUM"))
            tp_psum = ph_a.enter_context(
                tc.tile_pool(name="tp_psum", bufs=2, space="PSUM"))
            rope_pool = ph_a.enter_context(tc.tile_pool(name="rope", bufs=1))

            xt = []
            for dk in range(8):
                t = xt_pool.tile([128, N], BF16, name=f"xt{dk}", tag="xt")
                nc.sync.dma_start(
                    out=t, in_=x_p[:, dk * 128:(dk + 1) * 128], transpose=True)
                xt.append(t)
            nc.sync.dma_start(out=cc_sb, in_=cc_p[:])
            nc.sync.dma_start(out=ss_sb, in_=ss_p[:])

            # qkv + RoPE + vtok fused per column chunk, ordered so the
            # chunks needed by attention's first (qc2=0) pass finish first.
            dsts = (q_c, k_c, v_c)

            def rope_chunk(ch, pool, bufs):
                cs = slice(ch * 1024, (ch + 1) * 1024)
                for X in (q_c[ch], k_c[ch]):
                    xsw = pool.tile([128, 1024], BF16, name="xsw",
                                    tag="xsw", bufs=bufs)
                    for (d0, s0) in ((0, 32), (32, 0), (64, 96), (96, 64)):
                        nc.sync.dma_start(
                            out=xsw[d0:d0 + 32, :], in_=X[s0:s0 + 32, :])
                    a_t = pool.tile([128, 1024], BF16, name="ropea",
                                    tag="ropea", bufs=bufs)
                    nc.vector.tensor_mul(a_t, X, cc_sb[:, cs])
                    nc.vector.tensor_mul(xsw, xsw, ss_sb[:, cs])
                    nc.vector.tensor_add(X, a_t, xsw)

            def vtok_chunk(ch, psum_pool, psum_tag):
                nc.vector.memset(vt_c[ch][:, :, 64], 1.0)
                nc.vector.memset(vt_c[ch][:, :, 129], 1.0)
                for tj in range(8):
                    pst = psum_pool.tile([128, 128], BF16, name="tps",
                                         tag=psum_tag)
                    nc.tensor.transpose(
                        pst, v_c[ch][:, tj * 128:tj * 128 + 128], ident)
                    nc.vector.tensor_copy(vt_c[ch][:, tj, 0:64], pst[:, 0:64])
                    nc.vector.tensor_copy(vt_c[ch][:, tj, 65:129],
                                          pst[:, 64:128])

            for pi, ch in enumerate((0, 2, 1, 3)):
                # dk-outer accumulation across 6 live psum tiles so matmuls
                # interleave with the incoming xT transposes
                pss = [qk_psum.tile([128, 512], F32, name=f"qkps{t2}{r3}",
                                    tag="qkps")
                       for t2 in range(2) for r3 in range(3)]
                for i in range(8):
                    dk = (2 * pi + i) % 8
                    for r3 in range(3):
                        for t2 in range(2):
                            nc.tensor.matmul(
                                pss[t2 * 3 + r3],
                                w_sb[:, dk, r3 * 128:(r3 + 1) * 128],
                                xt[dk][:, (2 * ch + t2) * 512:
                                       (2 * ch + t2) * 512 + 512],
                                start=(i == 0), stop=(i == 7))
                for t2 in range(2):
                    for r3 in range(3):
                        dst = dsts[r3][ch][:, t2 * 512:t2 * 512 + 512]
                        if r3 == 2:
                            nc.vector.tensor_copy(dst, pss[t2 * 3 + r3])
                        else:
                            nc.scalar.copy(dst, pss[t2 * 3 + r3])
                if ch != 3:   # chunk 3 finalization deferred into attention
                    rope_chunk(ch, rope_pool, 3)
                    vtok_chunk(ch, tp_psum, "tps")

        ow_pool = top.enter_context(tc.tile_pool(name="ow", bufs=1))
        ow_sb = ow_pool.tile([128, 8, DM], BF16)
        nc.sync.dma_start(out=ow_sb, in_=ow_p.rearrange("(a p) r -> p a r", p=128))

        # ---- attention (with fused per-chunk normalization) ----
        ctxn = [ctxp.tile([64, N], BF16, name=f"ctxn{i}") for i in range(2)]
        a2a_in = [dram.tile([8, 64, 512], BF16, name=f"a2ai{i}")
                  for i in range(2)]
        a2a_out = [dram.tile([8, 64, 512], BF16, name=f"a2ao{i}")
                   for i in range(2)]
        with ExitStack() as ph_b:
            sc_psum = ph_b.enter_context(
                tc.tile_pool(name="sc_psum", bufs=2, space="PSUM"))
            ctx_psum = ph_b.enter_context(
                tc.tile_pool(name="ctx_psum", bufs=2, space="PSUM"))
            e_pool = ph_b.enter_context(tc.tile_pool(name="e_pool", bufs=3))
            w2_pool = ph_b.enter_context(tc.tile_pool(name="w2_pool", bufs=4))
            div_pool = ph_b.enter_context(tc.tile_pool(name="div", bufs=2))

            rope2 = ph_b.enter_context(tc.tile_pool(name="rope2", bufs=1))
            for hl in range(2):
                for qc2 in range(2):
                    h64 = hl * 64
                    # merged kerple window for all 16 k-tiles of this (qc2,hl):
                    # W3[p, j] = T[base + p + j], base = 1024*(1-qc2)
                    w3 = w2_pool.tile([128, 2960], BF16, name="w3", tag="w3")
                    base = hl * 4100 + 1024 * (1 - qc2)
                    src = bass.AP(tensor=kt_p, offset=base,
                                  ap=[[1, 128], [1, 2945]])
                    nc.sync.dma_start(out=w3[:, 0:2945], in_=src)
                    acc = [ctx_psum.tile([65, 1024], F32, name=f"acc{b}",
                                         tag="acc") for b in range(B)]
                    for kt_i in range(16):
                        if hl == 0 and qc2 == 0 and kt_i == 2:
                            rope_chunk(3, rope2, 1)
                            vtok_chunk(3, sc_psum, "scps")
                        k0 = kt_i * 128
                        kview = w3[:, 1024 + k0:k0:-1]
                        for b in range(B):
                            cb = b * S
                            ps = sc_psum.tile([128, 1024], F32, name="scps",
                                              tag="scps")
                            kc = k_c[2 * b + (k0 >= 1024)]
                            kcol = k0 % 1024
                            qc = q_c[2 * b + qc2]
                            for half in range(2):
                                nc.tensor.matmul(
                                    ps[:, half * 512:(half + 1) * 512],
                                    kc[h64:h64 + 64, kcol:kcol + 128],
                                    qc[h64:h64 + 64,
                                       half * 512:(half + 1) * 512],
                                    start=True, stop=True)
                            e = e_pool.tile([128, 1024], BF16, name="e", tag="e")
                            nc.scalar.activation(e, ps, Exp, scale=SCALE)
                            nc.vector.tensor_mul(e, e, kview)
                            gti = b * 16 + kt_i
                            vtt = vt_c[gti // 8][:, gti % 8,
                                                 hl * 65:hl * 65 + 65]
                            for half in range(2):
                                nc.tensor.matmul(
                                    acc[b][:, half * 512:(half + 1) * 512],
                                    vtt,
                                    e[:, half * 512:(half + 1) * 512],
                                    start=(kt_i == 0), stop=(kt_i == 15))
                    for b in range(B):
                        sl = slice(b * S + qc2 * 1024, b * S + qc2 * 1024 + 1024)
                        acc_sb = div_pool.tile([64, 1024], F32, name="accsb",
                                               tag="accsb", bufs=4)
                        den_t = div_pool.tile([1, 1024], F32, name="dent",
                                              tag="dent", bufs=4)
                        nc.scalar.copy(acc_sb, acc[b][0:64, :])
                        nc.scalar.copy(den_t, acc[b][64:65, :])
                        rcp = div_pool.tile([1, 1024], F32, name="rcp", tag="rcp")
                        nc.vector.reciprocal_approx_fast(rcp, den_t)
                        rb_sb = div_pool.tile([64, 1024], F32, name="rbsb",
                                              tag="rbsb")
                        nc.gpsimd.partition_broadcast(rb_sb, rcp)
                        nc.gpsimd.tensor_mul(ctxn[hl][:, sl], acc_sb, rb_sb)
                nc.sync.dma_start(
                    out=a2a_in[hl][:].transpose([1, 0, 2]),
                    in_=ctxn[hl].rearrange("p (j q) -> p j q", j=8))
                nc.gpsimd.collective_compute(
                    "AllToAll", mybir.AluOpType.bypass,
                    replica_groups=[list(range(8))],
                    ins=[a2a_in[hl].opt()], outs=[a2a_out[hl].opt()])


        # ---- output projection for this core's 512-token slice ----
        with ExitStack() as ph_y:
            y_psum = ph_y.enter_context(
                tc.tile_pool(name="y_psum", bufs=2, space="PSUM"))
            y_lhs = ph_y.enter_context(tc.tile_pool(name="y_lhs", bufs=8))
            y_out = ph_y.enter_context(tc.tile_pool(name="y_out", bufs=2))
            lhs = []
            for j in range(8):
                lh = y_lhs.tile([128, 512], BF16, name=f"ylh{j}", tag="ylh")
                nc.sync.dma_start(out=lh[0:64, :], in_=a2a_out[0][j])
                nc.sync.dma_start(out=lh[64:128, :], in_=a2a_out[1][j])
                lhs.append(lh)
            for tt in range(4):
                ps_y = y_psum.tile([128, 1024], F32, name="psy", tag="psy")
                for j in range(8):
                    for dmc in range(2):
                        nc.tensor.matmul(
                            ps_y[:, dmc * 512:(dmc + 1) * 512],
                            lhs[j][:, tt * 128:(tt + 1) * 128],
                            ow_sb[:, j, dmc * 512:(dmc + 1) * 512],
                            start=(j == 0), stop=(j == 7))
                y_sb = y_out.tile([128, 1024], F32, name="ysb", tag="ysb")
                nc.any.tensor_copy(y_sb, ps_y)
                nc.sync.dma_start(
                    out=out_p[tt * 128:(tt + 1) * 128, :], in_=y_sb)

    nc.compile()
    return nc


def _host_prep(x, qkv_w, out_w, bias_p, bias_a, rope_freqs, c):
    x = np.asarray(x, np.float32).reshape(N, DM)
    qkv_w = np.asarray(qkv_w, np.float32)
    out_w = np.asarray(out_w, np.float32)
    bias_p = np.asarray(bias_p, np.float32).reshape(H)
    bias_a = np.asarray(bias_a, np.float32).reshape(H)
    freqs = np.asarray(rope_freqs, np.float32)

    h0, h1 = 2 * c, 2 * c + 1
    rows = []
    for blk in range(3):
        for h in (h0, h1):
            rows.append(qkv_w[blk * 1024 + h * 64: blk * 1024 + h * 64 + 64])
    wT = np.ascontiguousarray(np.concatenate(rows, 0).T)

    pos = np.arange(S, dtype=np.float32)
    ang = pos[:, None] * freqs[None, :]
    cosT = np.cos(ang).T
    sinT = np.sin(ang).T
    cc64 = np.concatenate([cosT, cosT], 0)
    ss64 = np.concatenate([-sinT, sinT], 0)
    CC = np.tile(np.concatenate([cc64, cc64], 0), (1, B))
    SSg = np.tile(np.concatenate([ss64, ss64], 0), (1, B))

    p = np.maximum(bias_p, 0.01)
    a = np.maximum(bias_a, 0.01)
    idx = np.abs(np.arange(4100, dtype=np.float32) - 2048.0)
    ktab = np.stack([(1.0 + a[h] * idx) ** (-p[h]) for h in (h0, h1)], 0)

    return {
        "x_bf": x.astype(bf16),
        "wT": wT.astype(bf16),
        "out_wT": np.ascontiguousarray(out_w.T).astype(bf16),
        "cc": np.ascontiguousarray(CC).astype(bf16),
        "ssg": np.ascontiguousarray(SSg).astype(bf16),
        "ktab": np.ascontiguousarray(ktab).astype(bf16),
    }


def kernel(x, qkv_w, out_w, bias_p, bias_a, rope_freqs, _trace=False):
    from concourse.bass_utils import run_bass_kernel_spmd

    if "nc" not in _GRAPH_CACHE:
        _GRAPH_CACHE["nc"] = _build_graph()
    nc = _GRAPH_CACHE["nc"]

    in_maps = [
        _host_prep(x, qkv_w, out_w, bias_p, bias_a, rope_freqs, c)
        for c in range(8)
    ]
    res = run_bass_kernel_spmd(nc, in_maps, core_ids=list(range(8)),
                               trace=_trace)
    _GRAPH_CACHE["last_result"] = res
    y = np.concatenate([np.asarray(res.results[c]["out"]) for c in range(8)], 0)
    return np.ascontiguousarray(y.reshape(B, S, DM)).astype(np.float32)
